# revision 7
# baseline (speedup 1.0000x reference)
"""BiAttention Trainium2 kernel (mask-compacted fp16, permuted-n edition).

Reference math (per batch; n = m = 1024, d = 512):
    sim[n,m] = (x1*w3) @ x2.T + s1[n] + s2[m] + bias,  s1 = x1@w1, s2 = x2@w2
    row softmax over m with x2-masked columns dropped -> attn_a = P_row @ x2
    col softmax over n with x1-masked rows dropped    -> q2c = P_col.T @ x1
    attn_b = P_row @ q2c

The masks are ~50% dense, so masked positions are compacted away on the host:
  - m is compacted to m' = max unmasked x2 count (128-padded): every
    m-contraction (row softmax, attn_a, attn_b) skips masked columns.
  - n is PERMUTED so unmasked x1 rows come first (outputs are un-permuted on
    the host). The column softmax then only involves the leading n' rows, so
    its numerator EC = ET^T restricted to the first n' columns — obtained
    with cheap PE transposes of ET instead of a second matmul, and the
    q2c contraction shrinks to n' as well.

Kernel formulation (softmax is shift-invariant, so each direction only needs
the logit terms that vary along its own axis):
    ET[m',n] = exp(s3^T[m',n] + lane2[m']),  lane2 = s2 + bias + NEG*pad
        (s1[n] cancels in the row softmax; per-partition ACT exp bias)
    TC[n',m'] = ET^T (PE transpose of the leading n' column tiles)
    rowsum[n] = sum_m' ET[m',n]:
        n < n' : free via DVE accum_out on the TC PSUM->SBUF copies
        n >= n': tiny PE matmul chains against a ones vector
    colsum[m'] = sum_n' TC[n',m']*B[n'],  B = exp(s1)*keep1  (tiny PE chains
        against B; the x1 row mask and col-softmax exp(s1) factor live in B)
    attn_a = (ET.T @ x2') / rowsum
    q2c    = (TC.T @ (B*x1')) / colsum
    attn_b = (ET.T @ q2c) / rowsum
Padded m' columns have ET = 0 (lane2 = NEG), so their (finite, junk) q2c
rows are multiplied by exactly-0 weights in attn_b, matching the reference.
Trailing masked n rows have B = 0, so they drop out of colsum/q2c exactly.

Implementation notes:
  - exp() without max-subtraction: logits are O(+-5) and masking/padding is
    additive -30000 so exp underflows to exactly 0.
  - Matmuls run in fp16 (10-bit mantissa; 2-byte operands stream at 1
    cycle/row with fast-weight-load). PSUM accumulation is fp32.
  - All inputs are pre-swizzled on the host into the exact SBUF layout
    (fully contiguous per-partition lines -> fast DMA), and the loads are
    spread over five trigger queues (sync/gpsimd/vector/scalar).
  - Outputs are stored fp16 (halves the output DMA) and upcast + n-unpermuted
    on the host; |attn| <= ~6 so fp16 rounding adds ~5e-4 relative error.
  - Sharding: data-parallel over batch, 2 batches per core, 8 cores.
"""

import sys

import numpy as np

for _p in ("/opt/trn_rl_repo",):
    if _p not in sys.path:
        sys.path.append(_p)

import concourse.bass as bass
import concourse.mybir as mybir
import concourse.tile as tile
from concourse import bass_utils
from concourse.bass import ds, ts
from concourse.tile import ScopedClock

NCORES = 8
B, N, M, D = 16, 1024, 1024, 512
BPC = B // NCORES  # batches per core
NEG = -30000.0  # additive mask: exp(x + NEG) == 0 for |x| < ~100

F32 = mybir.dt.float32
F16 = mybir.dt.float16
MNP = np.float16
ADD = mybir.AluOpType.add

NT = N // 128  # 8 n-tiles (full axis: attn_a/attn_b output rows)
DC = D // 128  # 4 d-chunks
NH = N // 512  # 2 n-halves (PSUM-bank-sized slabs)

# ---------------------------------------------------------------------------
# Workarounds for this walrus build: at most ONE sync wait per instruction.
# ---------------------------------------------------------------------------

_ctr = [0]


def _split_multi_waits(nc):
    """Move extra sync waits onto same-engine InstNoOp carriers inserted
    immediately before the over-subscribed instruction."""
    for f in nc.m.functions:
        for bb in f.blocks:
            insts = bb.instructions
            i = 0
            while i < len(insts):
                inst = insts[i]
                si = getattr(inst, "sync_info", None)
                if si is not None and len(si.on_wait) > 1:
                    waits = list(si.on_wait)
                    carriers = []
                    for w in waits[:-1]:
                        _ctr[0] += 1
                        carriers.append(
                            mybir.InstNoOp(
                                name=f"I-waitsplit-{_ctr[0]}",
                                engine=inst.engine,
                                bass_nofuse=True,
                                sync_info=mybir.SyncInfo(on_wait=[w], on_update=[]),
                            )
                        )
                    inst.sync_info = mybir.SyncInfo(
                        on_wait=[waits[-1]], on_update=list(si.on_update)
                    )
                    insts[i:i] = carriers
                    i += len(carriers)
                i += 1


def _patched_drain_and_barrier(self, tick_clock, wait_clock):
    """TileContext tail drain: carry the global-clock waits on SP nops (the
    Drain opcode can't encode sync waits in this walrus build)."""
    nc = self.nc
    nop_inst = nc.sync.nop(nofuse=True)
    wait_clock.add_sem_waits(nop_inst.ins, ScopedClock({None: tick_clock.global_clock}))
    waits = list(nop_inst.ins.sync_info.on_wait)
    if len(waits) > 1:
        nop_inst.ins.sync_info = mybir.SyncInfo(on_wait=[waits[0]], on_update=[])
        for w in waits[1:]:
            extra = nc.sync.nop(nofuse=True)
            extra.ins.sync_info = mybir.SyncInfo(on_wait=[w], on_update=[])
    nc.sync.drain()
    nc.all_engine_barrier()
    assert self.sems is not None
    popped = nc._tile_sem_poison_stack.pop()
    assert popped is self._sem_poison
    nc.clear_and_free_semaphores(list(self.sems.allocated().values()))


tile.TileContext._drain_and_barrier = _patched_drain_and_barrier

# ---------------------------------------------------------------------------
# Kernel build
# ---------------------------------------------------------------------------

_cache = {}


def _build(MTp, NTp, split_waits=True):
    """MTp/NTp: number of 128-tiles of the compacted m'/permuted n' axes."""
    Mp, Np = MTp * 128, NTp * 128
    nc = bass.Bass("TRN2", target_bir_lowering=False, debug=False)

    # all inputs pre-swizzled to SBUF layout (contiguous partition lines)
    a1d = nc.dram_tensor("a1", [BPC, 2, 128, DC, 512], F16, kind="ExternalInput").ap()
    a2d = nc.dram_tensor("a2c", [BPC, 128, DC, Mp], F16, kind="ExternalInput").ap()
    x2cd = nc.dram_tensor("x2c", [BPC, 128, MTp, D], F16, kind="ExternalInput").ap()
    x1md = nc.dram_tensor("x1m", [BPC, 128, NTp, D], F16, kind="ExternalInput").ap()
    # lane2 = s2 + bias + NEG*pad, [128, MTp] per-partition layout (fp32)
    lvecd = nc.dram_tensor("lvec", [BPC, 128, MTp], F32, kind="ExternalInput").ap()
    # colsum weights B = exp(s1)*keep1, [128, NTp] per-partition layout (fp16)
    bvd = nc.dram_tensor("bv", [BPC, 128, NTp], F16, kind="ExternalInput").ap()
    idd = nc.dram_tensor("ident", [128, 128], F16, kind="ExternalInput").ap()
    oad = nc.dram_tensor("attn_a", [BPC, N, D], F16, kind="ExternalOutput").ap()
    obd = nc.dram_tensor("attn_b", [BPC, N, D], F16, kind="ExternalOutput").ap()

    EXP = mybir.ActivationFunctionType.Exp

    with tile.TileContext(nc) as tc:
        with (
            tc.tile_pool(name="xin", bufs=2) as xin,
            tc.tile_pool(name="amat", bufs=2) as amat,
            tc.tile_pool(name="emat", bufs=2) as emat,
            tc.tile_pool(name="qmat", bufs=2) as qmat,
            tc.tile_pool(name="small", bufs=2) as small,
            tc.tile_pool(name="ostage", bufs=4) as ostage,
            tc.tile_pool(name="mm_ps", bufs=4, space="PSUM") as mm_ps,
            tc.tile_pool(name="acc_ps", bufs=2, space="PSUM") as acc_ps,
            tc.tile_pool(name="sm_ps", bufs=2, space="PSUM") as sm_ps,
        ):
            # trigger the ACT exp table load while the first DMAs are in
            # flight (the first real exp would otherwise pay ~2.7us mid-loop)
            warm = small.tile([128, 2], F32, tag="warm")
            nc.vector.memset(warm[:], 0.0)
            nc.scalar.activation(out=warm[:], in_=warm[:], func=EXP)
            # warm the PE clock (HAM) with dummy matmuls during the load
            # wait; without this the first ~3.4us of real matmuls run at
            # half clock
            wsb = small.tile([128, 512], F16, tag="wsb")
            nc.vector.memset(wsb[:], 0.0)
            wps = mm_ps.tile([128, 512], F32, tag="mm")
            for _ in range(8):
                nc.tensor.matmul(wps[:], wsb[:, 0:128], wsb[:], start=True, stop=True)
            ones = small.tile([128, 1], F16, tag="ones")
            nc.vector.memset(ones[:], 1.0)
            ident = small.tile([128, 128], F16, tag="ident")
            nc.sync.dma_start(out=ident[:], in_=idd)

            for b in range(BPC):
                # ---- loads (pre-swizzled; one contiguous block each) ------
                lvec = small.tile([128, MTp], F32, tag="lvec")
                nc.sync.dma_start(out=lvec[:], in_=lvecd[b])
                bvt = small.tile([128, NTp], F16, tag="bvt")
                nc.sync.dma_start(out=bvt[:], in_=bvd[b])
                A1 = amat.tile([128, NH, DC, 512], F16, tag="A1")  # w3*x1^T
                A2 = amat.tile([128, DC, Mp], F16, tag="A2")  # x2^T compact
                nc.gpsimd.dma_start(out=A1[:, 0], in_=a1d[b, 0])
                nc.sync.dma_start(out=A2[:], in_=a2d[b])
                nc.scalar.dma_start(out=A1[:, 1], in_=a1d[b, 1])
                X2C = xin.tile([128, MTp, D], F16, tag="X2C")
                X1M = xin.tile([128, NTp, D], F16, tag="X1M")
                nc.sync.dma_start(out=X2C[:], in_=x2cd[b])
                nc.gpsimd.dma_start(out=X1M[:], in_=x1md[b])

                # ---- ET = exp(s3^T + lane2[m'])  [m'-part, n-free] --------
                ET = emat.tile([128, MTp, N], F16, tag="ET")
                for nh in range(NH):
                    for mt in range(MTp):
                        ps = mm_ps.tile([128, 512], F32, tag="mm")
                        for c in range(DC):
                            nc.tensor.matmul(
                                ps[:],
                                A2[:, c, ts(mt, 128)],
                                A1[:, nh, c, :],
                                start=(c == 0),
                                stop=(c == DC - 1),
                            )
                        nc.scalar.activation(
                            out=ET[:, mt, ds(512 * nh, 512)],
                            in_=ps[:],
                            func=EXP,
                            bias=lvec[:, mt : mt + 1],
                        )

                # ---- TC = ET^T for the leading n' tiles (PE transpose) ----
                # The DVE copy off PSUM also reduces each row into rtmp:
                # rowsum[n] = sum_m' TC[n, m'] for free (accum_out).
                TC = emat.tile([128, NTp, Mp], F16, tag="TC")
                rtmp = small.tile([128, NT], F32, tag="rtmp")
                RR = small.tile([128, NT], F32, tag="RR")
                for nt in range(NTp):
                    tps = mm_ps.tile([128, Mp], F16, tag="mm")
                    for mt in range(MTp):
                        nc.tensor.transpose(
                            tps[:, ts(mt, 128)],
                            ET[:, mt, ts(nt, 128)],
                            ident[:],
                        )
                    nc.vector.tensor_scalar(
                        out=TC[:, nt, :],
                        in0=tps[:],
                        scalar1=0.0,
                        scalar2=None,
                        op0=ADD,
                        op1=ADD,
                        accum_out=rtmp[:, nt : nt + 1],
                    )
                # rowsum for the trailing (masked-n) tiles: tiny PE chains
                for nt in range(NTp, NT):
                    rps = sm_ps.tile([128, 1], F32, tag="sm")
                    for mc in range(MTp):
                        nc.tensor.matmul(
                            rps[:],
                            ET[:, mc, ts(nt, 128)],
                            ones[:],
                            start=(mc == 0),
                            stop=(mc == MTp - 1),
                        )
                    nc.vector.tensor_copy(rtmp[:, nt : nt + 1], rps[:])
                nc.vector.reciprocal(RR[:], rtmp[:])

                # ---- attn_a = (ET.T @ x2') / rowsum -----------------------
                for nt in range(NT):
                    aps = acc_ps.tile([128, 512], F32, tag="acc")
                    for mc in range(MTp):
                        nc.tensor.matmul(
                            aps[:],
                            ET[:, mc, ts(nt, 128)],
                            X2C[:, mc, :],
                            start=(mc == 0),
                            stop=(mc == MTp - 1),
                        )
                    stage = ostage.tile([128, 512], F16, tag="stage")
                    nc.vector.tensor_scalar_mul(stage[:], aps[:], RR[:, nt : nt + 1])
                    nc.sync.dma_start(out=oad[b, ts(nt, 128), :], in_=stage[:])

                # ---- colsum[m'] = sum_n' TC*B (tiny PE chains vs B) -------
                ctmp = small.tile([128, MTp], F32, tag="ctmp")
                CR = small.tile([128, MTp], F32, tag="CR")
                for mt in range(MTp):
                    cps = sm_ps.tile([128, 1], F32, tag="sm")
                    for nc_ in range(NTp):
                        nc.tensor.matmul(
                            cps[:],
                            TC[:, nc_, ts(mt, 128)],
                            bvt[:, nc_ : nc_ + 1],
                            start=(nc_ == 0),
                            stop=(nc_ == NTp - 1),
                        )
                    nc.vector.tensor_scalar_add(ctmp[:, mt : mt + 1], cps[:], 1e-30)
                nc.vector.reciprocal(CR[:], ctmp[:])

                # ---- q2c = (TC.T @ (B*x1')) / colsum ----------------------
                Q2C = qmat.tile([128, MTp, D], F16, tag="Q2C")
                for mt in range(MTp):
                    qps = acc_ps.tile([128, 512], F32, tag="acc")
                    for nc_ in range(NTp):
                        nc.tensor.matmul(
                            qps[:],
                            TC[:, nc_, ts(mt, 128)],
                            X1M[:, nc_, :],
                            start=(nc_ == 0),
                            stop=(nc_ == NTp - 1),
                        )
                    nc.scalar.mul(Q2C[:, mt, :], qps[:], CR[:, mt : mt + 1])

                # ---- attn_b = (ET.T @ q2c) / rowsum -----------------------
                for nt in range(NT):
                    bps = acc_ps.tile([128, 512], F32, tag="acc")
                    for mc in range(MTp):
                        nc.tensor.matmul(
                            bps[:],
                            ET[:, mc, ts(nt, 128)],
                            Q2C[:, mc, :],
                            start=(mc == 0),
                            stop=(mc == MTp - 1),
                        )
                    stage = ostage.tile([128, 512], F16, tag="stage")
                    nc.vector.tensor_scalar_mul(stage[:], bps[:], RR[:, nt : nt + 1])
                    nc.scalar.dma_start(out=obd[b, ts(nt, 128), :], in_=stage[:])

    if split_waits:
        _split_multi_waits(nc)
    return nc


def _get_nc(MTp, NTp):
    key = (MTp, NTp)
    if key not in _cache:
        _cache[key] = _build(MTp, NTp)
    return _cache[key]


# ---------------------------------------------------------------------------
# Host entry point
# ---------------------------------------------------------------------------


def _prep(x1, x1_mask, x2, x2_mask, w, bias):
    """Host-side marshaling: n-permutation, m-compaction gathers, SBUF-layout
    swizzles, fp16 casts, and the tiny per-row/col logit vectors."""
    x1 = np.asarray(x1, dtype=np.float32)
    x2 = np.asarray(x2, dtype=np.float32)
    x1_mask = np.asarray(x1_mask, dtype=bool)
    x2_mask = np.asarray(x2_mask, dtype=bool)
    w = np.asarray(w, dtype=np.float32)
    bias_f = float(np.asarray(bias, dtype=np.float32))

    d = x1.shape[-1]
    w1, w2, w3 = w[:d], w[d : 2 * d], w[2 * d :]
    s1 = np.einsum("bnd,d->bn", x1, w1)
    s2 = np.einsum("bmd,d->bm", x2, w2)
    x1w3 = x1 * w3

    keepm = [np.flatnonzero(~x2_mask[b]) for b in range(B)]
    MTp = max(1, -(-max(len(i) for i in keepm) // 128))
    NTp = max(1, -(-int((~x1_mask).sum(1).max()) // 128))
    Mp, Np = MTp * 128, NTp * 128

    # n permutation: unmasked rows first (stable)
    perm = np.argsort(x1_mask, axis=1, kind="stable")  # [B, N]

    a1 = np.zeros((B, 2, 128, DC, 512), dtype=MNP)
    a2c = np.zeros((B, 128, DC, Mp), dtype=MNP)
    x2c = np.zeros((B, 128, MTp, D), dtype=MNP)
    x1m = np.zeros((B, 128, NTp, D), dtype=MNP)
    lane2c = np.full((B, Mp), NEG, dtype=np.float32)
    bvf = np.zeros((B, Np), dtype=np.float32)
    for b in range(B):
        im, pi = keepm[b], perm[b]
        # a1: (w3*x1)^T with permuted n, swizzled [half, p, c, 512]
        a1t = x1w3[b][pi].T.astype(MNP)  # [D, N]
        a1[b] = (
            a1t.reshape(DC, 128, 2, 512).transpose(2, 1, 0, 3)
        )
        a2t = np.zeros((d, Mp), dtype=MNP)
        a2t[:, : len(im)] = x2[b, im].T
        a2c[b] = a2t.reshape(DC, 128, Mp).transpose(1, 0, 2)
        xc = np.zeros((Mp, d), dtype=MNP)
        xc[: len(im)] = x2[b, im]
        x2c[b] = xc.reshape(MTp, 128, d).transpose(1, 0, 2)
        bvp = np.exp(s1[b][pi[:Np]]) * (~x1_mask[b][pi[:Np]])
        bvf[b] = bvp
        xm = (x1[b][pi[:Np]] * bvp[:, None]).astype(MNP)
        x1m[b] = xm.reshape(NTp, 128, d).transpose(1, 0, 2)
        lane2c[b, : len(im)] = s2[b, im] + bias_f

    def ptile(v, dt):  # [B, T*128] -> [B, 128, T]
        return np.ascontiguousarray(
            v.reshape(B, -1, 128).transpose(0, 2, 1).astype(dt)
        )

    tensors = {
        "a1": a1,
        "a2c": a2c,
        "x2c": x2c,
        "x1m": x1m,
        "lvec": ptile(lane2c, np.float32),
        "bv": ptile(bvf, MNP),
        "ident": np.tile(np.eye(128, dtype=MNP)[None], (BPC, 1, 1)),
    }
    return tensors, MTp, NTp, perm


def _unpermute(res_a, res_b, perm):
    attn_a = np.empty((B, N, D), dtype=np.float32)
    attn_b = np.empty((B, N, D), dtype=np.float32)
    for b in range(B):
        attn_a[b, perm[b]] = res_a[b].astype(np.float32)
        attn_b[b, perm[b]] = res_b[b].astype(np.float32)
    return attn_a, attn_b


def _run(x1, x1_mask, x2, x2_mask, w, bias, **run_kwargs):
    full, MTp, NTp, perm = _prep(x1, x1_mask, x2, x2_mask, w, bias)
    nc = _get_nc(MTp, NTp)
    ident = full.pop("ident")[0]
    in_maps = []
    for core in range(NCORES):
        lo, hi = core * BPC, (core + 1) * BPC
        m = {k: v[lo:hi] for k, v in full.items()}
        m["ident"] = ident
        in_maps.append(m)
    res = bass_utils.run_bass_kernel_spmd(
        nc, in_maps, core_ids=list(range(NCORES)), **run_kwargs
    )
    res_a = np.concatenate([res.results[c]["attn_a"] for c in range(NCORES)], axis=0)
    res_b = np.concatenate([res.results[c]["attn_b"] for c in range(NCORES)], axis=0)
    attn_a, attn_b = _unpermute(res_a, res_b, perm)
    return (attn_a, attn_b), res


def kernel(x1, x1_mask, x2, x2_mask, w, bias):
    out, _ = _run(x1, x1_mask, x2, x2_mask, w, bias)
    return out


# revision 11
# speedup vs baseline: 1.1092x; 1.1092x over previous
"""BiAttention Trainium2 kernel (mask-compacted fp16, permuted-n, flat-DMA).

Reference math (per batch; n = m = 1024, d = 512):
    sim[n,m] = (x1*w3) @ x2.T + s1[n] + s2[m] + bias,  s1 = x1@w1, s2 = x2@w2
    row softmax over m with x2-masked columns dropped -> attn_a = P_row @ x2
    col softmax over n with x1-masked rows dropped    -> q2c = P_col.T @ x1
    attn_b = P_row @ q2c

The masks are ~50% dense, so masked positions are compacted away on the host:
  - m is compacted to m' = max unmasked x2 count (128-padded): every
    m-contraction (row softmax, attn_a, attn_b) skips masked columns.
  - n is PERMUTED so unmasked x1 rows come first (outputs are un-permuted on
    the host). The column softmax then only involves the leading n' rows, so
    its numerator EC = ET^T restricted to the first n' columns — obtained
    with cheap PE transposes of ET instead of a second matmul, and the
    q2c contraction shrinks to n' as well.

Kernel formulation (softmax is shift-invariant, so each direction only needs
the logit terms that vary along its own axis):
    ET[m',n] = exp(s3^T[m',n] + lane2[m']),  lane2 = s2 + bias + NEG*pad
        (s1[n] cancels in the row softmax; per-partition ACT exp bias)
    TC[n',m'] = ET^T (PE transpose of the leading n' column tiles)
    rowsum[n] = sum_m' ET[m',n]:
        n < n' : free via DVE accum_out on the TC PSUM->SBUF copies
        n >= n': tiny PE matmul chains against a ones vector
    colsum[m'] = sum_n' TC[n',m']*B[n'],  B = exp(s1)*keep1  (tiny PE chains
        against B; the x1 row mask and col-softmax exp(s1) factor live in B)
    attn_a = (ET.T @ x2') / rowsum
    q2c    = (TC.T @ (B*x1')) / colsum
    attn_b = (ET.T @ q2c) / rowsum
Padded m' columns have ET = 0 (lane2 = NEG), so their (finite, junk) q2c
rows are multiplied by exactly-0 weights in attn_b, matching the reference.
Trailing masked n rows have B = 0, so they drop out of colsum/q2c exactly.

Implementation notes:
  - exp() without max-subtraction: logits are O(+-5) and masking/padding is
    additive -30000 so exp underflows to exactly 0 (exactly representable
    in the fp16 lane2 vector; its ~1e-3 rounding on live logits is far
    inside the error budget).
  - Matmuls run in fp16. PSUM accumulation is fp32.
  - DMA is descriptor-rate limited (~15ns/descriptor, serialized per
    queue), so every transfer is exactly 128 contiguous partition lines:
    inputs are pre-swizzled flat on the host, the small per-row vectors
    (lane2, B) ride as extra columns of the A2 pack, the identity matrix
    rides in the A1 pack, and outputs are staged into one [128, N/128*D]
    tile per output and shipped as two half-tensor DMAs.
  - Outputs fp16 (halves output DMA), upcast + un-permuted on host.
  - Sharding: data-parallel over batch, 2 batches per core, 8 cores.
"""

import sys

import numpy as np

for _p in ("/opt/trn_rl_repo",):
    if _p not in sys.path:
        sys.path.append(_p)

import concourse.bass as bass
import concourse.mybir as mybir
import concourse.tile as tile
from concourse import bass_utils
from concourse.bass import ds, ts
from concourse.tile import ScopedClock

NCORES = 8
B, N, M, D = 16, 1024, 1024, 512
BPC = B // NCORES  # batches per core
NEG = -30000.0  # additive mask: exp(x + NEG) == 0 for |x| < ~100

F32 = mybir.dt.float32
F16 = mybir.dt.float16
MNP = np.float16
ADD = mybir.AluOpType.add

NT = N // 128  # 8 n-tiles (full axis: attn_a/attn_b output rows)
DC = D // 128  # 4 d-chunks
NH = N // 512  # 2 n-halves (PSUM-bank-sized slabs)

# ---------------------------------------------------------------------------
# Workarounds for this walrus build: at most ONE sync wait per instruction.
# ---------------------------------------------------------------------------

_ctr = [0]


def _split_multi_waits(nc):
    """Move extra sync waits onto same-engine InstNoOp carriers inserted
    immediately before the over-subscribed instruction."""
    for f in nc.m.functions:
        for bb in f.blocks:
            insts = bb.instructions
            i = 0
            while i < len(insts):
                inst = insts[i]
                si = getattr(inst, "sync_info", None)
                if si is not None and len(si.on_wait) > 1:
                    waits = list(si.on_wait)
                    carriers = []
                    for w in waits[:-1]:
                        _ctr[0] += 1
                        carriers.append(
                            mybir.InstNoOp(
                                name=f"I-waitsplit-{_ctr[0]}",
                                engine=inst.engine,
                                bass_nofuse=True,
                                sync_info=mybir.SyncInfo(on_wait=[w], on_update=[]),
                            )
                        )
                    inst.sync_info = mybir.SyncInfo(
                        on_wait=[waits[-1]], on_update=list(si.on_update)
                    )
                    insts[i:i] = carriers
                    i += len(carriers)
                i += 1


def _patched_drain_and_barrier(self, tick_clock, wait_clock):
    """TileContext tail drain: carry the global-clock waits on SP nops (the
    Drain opcode can't encode sync waits in this walrus build)."""
    nc = self.nc
    nop_inst = nc.sync.nop(nofuse=True)
    wait_clock.add_sem_waits(nop_inst.ins, ScopedClock({None: tick_clock.global_clock}))
    waits = list(nop_inst.ins.sync_info.on_wait)
    if len(waits) > 1:
        nop_inst.ins.sync_info = mybir.SyncInfo(on_wait=[waits[0]], on_update=[])
        for w in waits[1:]:
            extra = nc.sync.nop(nofuse=True)
            extra.ins.sync_info = mybir.SyncInfo(on_wait=[w], on_update=[])
    nc.sync.drain()
    nc.all_engine_barrier()
    assert self.sems is not None
    popped = nc._tile_sem_poison_stack.pop()
    assert popped is self._sem_poison
    nc.clear_and_free_semaphores(list(self.sems.allocated().values()))


tile.TileContext._drain_and_barrier = _patched_drain_and_barrier

# ---------------------------------------------------------------------------
# Kernel build
# ---------------------------------------------------------------------------

_cache = {}


def _build(MTp, NTp, split_waits=True):
    """MTp/NTp: number of 128-tiles of the compacted m'/permuted n' axes."""
    Mp, Np = MTp * 128, NTp * 128
    # flat pack geometry
    A1W = NH * DC * 512 + 128  # [h0 (2048) | ident (128) | h1 (2048)]
    H0 = 0  # h0 block offset
    IDO = DC * 512  # ident offset (2048)
    H1 = DC * 512 + 128  # h1 block offset (2176)
    A2W = DC * Mp + MTp + NTp  # [x2^T chunks | lane2 | B]
    LNO = DC * Mp  # lane2 offset
    BVO = DC * Mp + MTp  # B offset
    OW = NT * D  # flat output width per partition

    nc = bass.Bass("TRN2", target_bir_lowering=False, debug=False)

    a1h0d = nc.dram_tensor("a1h0", [BPC, 128, H1], F16, kind="ExternalInput").ap()
    a1h1d = nc.dram_tensor("a1h1", [BPC, 128, A1W - H1], F16, kind="ExternalInput").ap()
    a2d = nc.dram_tensor("a2p", [BPC, 128, A2W], F16, kind="ExternalInput").ap()
    x2cd = nc.dram_tensor("x2c", [BPC, 128, MTp * D], F16, kind="ExternalInput").ap()
    x1md = nc.dram_tensor("x1m", [BPC, 128, NTp * D], F16, kind="ExternalInput").ap()
    oad = nc.dram_tensor("attn_a", [BPC, 128, OW], F16, kind="ExternalOutput").ap()
    obd = nc.dram_tensor("attn_b", [BPC, 128, OW], F16, kind="ExternalOutput").ap()

    EXP = mybir.ActivationFunctionType.Exp

    with tile.TileContext(nc) as tc:
        with (
            tc.tile_pool(name="xin", bufs=2) as xin,
            tc.tile_pool(name="amat", bufs=2) as amat,
            tc.tile_pool(name="emat", bufs=2) as emat,
            tc.tile_pool(name="qmat", bufs=2) as qmat,
            tc.tile_pool(name="small", bufs=2) as small,
            tc.tile_pool(name="ostage", bufs=2) as ostage,
            tc.tile_pool(name="mm_ps", bufs=4, space="PSUM") as mm_ps,
            tc.tile_pool(name="acc_ps", bufs=2, space="PSUM") as acc_ps,
            tc.tile_pool(name="sm_ps", bufs=2, space="PSUM") as sm_ps,
        ):
            # trigger the ACT exp table load while the first DMAs are in
            # flight (the first real exp would otherwise pay ~2.7us mid-loop)
            warm = small.tile([128, 2], F32, tag="warm")
            nc.vector.memset(warm[:], 0.0)
            nc.scalar.activation(out=warm[:], in_=warm[:], func=EXP)
            # warm the PE clock (HAM) with dummy matmuls during the load
            # wait; without this the first ~3.4us of real matmuls run at
            # half clock
            wsb = small.tile([128, 512], F16, tag="wsb")
            nc.vector.memset(wsb[:], 0.0)
            wps = mm_ps.tile([128, 512], F32, tag="mm")
            for _ in range(8):
                nc.tensor.matmul(wps[:], wsb[:, 0:128], wsb[:], start=True, stop=True)
            ones = small.tile([128, 1], F16, tag="ones")
            nc.vector.memset(ones[:], 1.0)
            epsv = small.tile([128, 1], F32, tag="epsv")
            nc.vector.memset(epsv[:], 1e-30)

            for b in range(BPC):
                # ---- loads: one contiguous 128-line transfer each ---------
                A1 = amat.tile([128, A1W], F16, tag="A1")  # w3*x1^T | ident
                A2 = amat.tile([128, A2W], F16, tag="A2")  # x2^T | lane2 | B
                nc.sync.dma_start(out=A2[:], in_=a2d[b])
                nc.scalar.dma_start(out=A1[:, 0:H1], in_=a1h0d[b])
                nc.scalar.dma_start(out=A1[:, H1:A1W], in_=a1h1d[b])
                X2C = xin.tile([128, MTp * D], F16, tag="X2C")
                X1M = xin.tile([128, NTp * D], F16, tag="X1M")
                nc.sync.dma_start(out=X2C[:], in_=x2cd[b])
                nc.gpsimd.dma_start(out=X1M[:], in_=x1md[b])
                ident = A1[:, ds(IDO, 128)]

                def a1s(nh, c):
                    return A1[:, ds((H1 if nh else H0) + c * 512, 512)]

                # ---- ET = exp(s3^T + lane2[m'])  [m'-part, n-free] --------
                ET = emat.tile([128, MTp, N], F16, tag="ET")
                for nh in range(NH):
                    for mt in range(MTp):
                        ps = mm_ps.tile([128, 512], F32, tag="mm")
                        for c in range(DC):
                            nc.tensor.matmul(
                                ps[:],
                                A2[:, ds(c * Mp + mt * 128, 128)],
                                a1s(nh, c),
                                start=(c == 0),
                                stop=(c == DC - 1),
                            )
                        nc.scalar.activation(
                            out=ET[:, mt, ds(512 * nh, 512)],
                            in_=ps[:],
                            func=EXP,
                            bias=A2[:, ds(LNO + mt, 1)],
                        )

                # ---- TC = ET^T for the leading n' tiles (PE transpose) ----
                # The DVE copy off PSUM also reduces each row into rtmp:
                # rowsum[n] = sum_m' TC[n, m'] for free (accum_out).
                TC = emat.tile([128, NTp, Mp], F16, tag="TC")
                rtmp = small.tile([128, NT], F32, tag="rtmp")
                RR = small.tile([128, NT], F32, tag="RR")
                for nt in range(NTp):
                    tps = mm_ps.tile([128, Mp], F16, tag="mm")
                    for mt in range(MTp):
                        nc.tensor.transpose(
                            tps[:, ts(mt, 128)],
                            ET[:, mt, ts(nt, 128)],
                            ident,
                        )
                    nc.vector.tensor_scalar(
                        out=TC[:, nt, :],
                        in0=tps[:],
                        scalar1=0.0,
                        scalar2=None,
                        op0=ADD,
                        op1=ADD,
                        accum_out=rtmp[:, nt : nt + 1],
                    )
                # rowsum for the trailing (masked-n) tiles: tiny PE chains
                for nt in range(NTp, NT):
                    rps = sm_ps.tile([128, 1], F32, tag="sm")
                    for mc in range(MTp):
                        nc.tensor.matmul(
                            rps[:],
                            ET[:, mc, ts(nt, 128)],
                            ones[:],
                            start=(mc == 0),
                            stop=(mc == MTp - 1),
                        )
                    nc.vector.tensor_copy(rtmp[:, nt : nt + 1], rps[:])
                nc.vector.reciprocal(RR[:], rtmp[:])

                # ---- attn_a = (ET.T @ x2') / rowsum -----------------------
                SA = ostage.tile([128, OW], F16, tag="sa")
                for nt in range(NT):
                    aps = acc_ps.tile([128, 512], F32, tag="acc")
                    for mc in range(MTp):
                        nc.tensor.matmul(
                            aps[:],
                            ET[:, mc, ts(nt, 128)],
                            X2C[:, ds(mc * D, D)],
                            start=(mc == 0),
                            stop=(mc == MTp - 1),
                        )
                    nc.vector.tensor_scalar_mul(
                        SA[:, ds(nt * D, D)], aps[:], RR[:, nt : nt + 1]
                    )
                    if nt == NT // 2 - 1:
                        nc.sync.dma_start(
                            out=oad[b][:, 0 : OW // 2], in_=SA[:, 0 : OW // 2]
                        )
                nc.sync.dma_start(out=oad[b][:, OW // 2 : OW], in_=SA[:, OW // 2 : OW])

                # ---- colsum[m'] = sum_n' TC*B (tiny PE chains vs B) -------
                # epilogue adds run on ACT: the DVE queue is busy with the
                # attn_a stage scalings and would serialize the chains.
                ctmp = small.tile([128, MTp], F32, tag="ctmp")
                CR = small.tile([128, MTp], F32, tag="CR")
                for mt in range(MTp):
                    cps = sm_ps.tile([128, 1], F32, tag="sm")
                    for nc_ in range(NTp):
                        nc.tensor.matmul(
                            cps[:],
                            TC[:, nc_, ts(mt, 128)],
                            A2[:, ds(BVO + nc_, 1)],
                            start=(nc_ == 0),
                            stop=(nc_ == NTp - 1),
                        )
                    nc.scalar.activation(
                        out=ctmp[:, mt : mt + 1],
                        in_=cps[:],
                        func=mybir.ActivationFunctionType.Identity,
                        bias=epsv[:],
                    )
                nc.vector.reciprocal(CR[:], ctmp[:])

                # ---- q2c = (TC.T @ (B*x1')) / colsum ----------------------
                Q2C = qmat.tile([128, MTp, D], F16, tag="Q2C")
                for mt in range(MTp):
                    qps = acc_ps.tile([128, 512], F32, tag="acc")
                    for nc_ in range(NTp):
                        nc.tensor.matmul(
                            qps[:],
                            TC[:, nc_, ts(mt, 128)],
                            X1M[:, ds(nc_ * D, D)],
                            start=(nc_ == 0),
                            stop=(nc_ == NTp - 1),
                        )
                    nc.scalar.mul(Q2C[:, mt, :], qps[:], CR[:, mt : mt + 1])

                # ---- attn_b = (ET.T @ q2c) / rowsum -----------------------
                SB = ostage.tile([128, OW], F16, tag="sb")
                for nt in range(NT):
                    bps = acc_ps.tile([128, 512], F32, tag="acc")
                    for mc in range(MTp):
                        nc.tensor.matmul(
                            bps[:],
                            ET[:, mc, ts(nt, 128)],
                            Q2C[:, mc, :],
                            start=(mc == 0),
                            stop=(mc == MTp - 1),
                        )
                    nc.vector.tensor_scalar_mul(
                        SB[:, ds(nt * D, D)], bps[:], RR[:, nt : nt + 1]
                    )
                    if nt == NT // 2 - 1:
                        nc.scalar.dma_start(
                            out=obd[b][:, 0 : OW // 2], in_=SB[:, 0 : OW // 2]
                        )
                nc.scalar.dma_start(
                    out=obd[b][:, OW // 2 : OW], in_=SB[:, OW // 2 : OW]
                )

    if split_waits:
        _split_multi_waits(nc)
    return nc


def _get_nc(MTp, NTp):
    key = (MTp, NTp)
    if key not in _cache:
        _cache[key] = _build(MTp, NTp)
    return _cache[key]


# ---------------------------------------------------------------------------
# Host entry point
# ---------------------------------------------------------------------------


def _prep(x1, x1_mask, x2, x2_mask, w, bias):
    """Host-side marshaling: n-permutation, m-compaction gathers, flat
    SBUF-layout swizzles, fp16 casts, and the fused per-row/col vectors."""
    x1 = np.asarray(x1, dtype=np.float32)
    x2 = np.asarray(x2, dtype=np.float32)
    x1_mask = np.asarray(x1_mask, dtype=bool)
    x2_mask = np.asarray(x2_mask, dtype=bool)
    w = np.asarray(w, dtype=np.float32)
    bias_f = float(np.asarray(bias, dtype=np.float32))

    d = x1.shape[-1]
    w1, w2, w3 = w[:d], w[d : 2 * d], w[2 * d :]
    s1 = np.einsum("bnd,d->bn", x1, w1)
    s2 = np.einsum("bmd,d->bm", x2, w2)
    x1w3 = x1 * w3

    keepm = [np.flatnonzero(~x2_mask[b]) for b in range(B)]
    MTp = max(1, -(-max(len(i) for i in keepm) // 128))
    NTp = max(1, -(-int((~x1_mask).sum(1).max()) // 128))
    Mp, Np = MTp * 128, NTp * 128
    H1 = DC * 512 + 128
    A2W = DC * Mp + MTp + NTp

    perm = np.argsort(x1_mask, axis=1, kind="stable")  # unmasked n first

    a1h0 = np.zeros((B, 128, H1), dtype=MNP)
    a1h1 = np.zeros((B, 128, DC * 512), dtype=MNP)
    a2p = np.zeros((B, 128, A2W), dtype=MNP)
    x2c = np.zeros((B, 128, MTp * D), dtype=MNP)
    x1m = np.zeros((B, 128, NTp * D), dtype=MNP)
    eye = np.eye(128, dtype=MNP)
    for b in range(B):
        im, pi = keepm[b], perm[b]
        a1t = x1w3[b][pi].T.astype(MNP)  # [D, N] permuted n
        # [DC, 128, NH, 512] -> per half [128, DC*512]
        a1q = a1t.reshape(DC, 128, NH, 512)
        a1h0[b, :, : DC * 512] = (
            a1q[:, :, 0].transpose(1, 0, 2).reshape(128, DC * 512)
        )
        a1h0[b, :, DC * 512 :] = eye
        a1h1[b] = a1q[:, :, 1].transpose(1, 0, 2).reshape(128, DC * 512)
        a2t = np.zeros((d, Mp), dtype=MNP)
        a2t[:, : len(im)] = x2[b, im].T
        a2p[b, :, : DC * Mp] = a2t.reshape(DC, 128, Mp).transpose(1, 0, 2).reshape(
            128, DC * Mp
        )
        lane2 = np.full(Mp, NEG, dtype=np.float32)
        lane2[: len(im)] = s2[b, im] + bias_f
        a2p[b, :, DC * Mp : DC * Mp + MTp] = lane2.reshape(MTp, 128).T.astype(MNP)
        bvp = np.exp(s1[b][pi[:Np]]) * (~x1_mask[b][pi[:Np]])
        a2p[b, :, DC * Mp + MTp :] = bvp.reshape(NTp, 128).T.astype(MNP)
        xc = np.zeros((Mp, d), dtype=MNP)
        xc[: len(im)] = x2[b, im]
        x2c[b] = xc.reshape(MTp, 128, d).transpose(1, 0, 2).reshape(128, MTp * d)
        xm = (x1[b][pi[:Np]] * bvp[:, None]).astype(MNP)
        x1m[b] = xm.reshape(NTp, 128, d).transpose(1, 0, 2).reshape(128, NTp * d)

    tensors = {
        "a1h0": a1h0,
        "a1h1": a1h1,
        "a2p": a2p,
        "x2c": x2c,
        "x1m": x1m,
    }
    return tensors, MTp, NTp, perm


def _unpermute(res_a, res_b, perm):
    # res [B, 128, NT*D] -> [B, N, D] (row n' = nt*128+p), then un-permute n
    def unswz(r):
        return (
            r.reshape(B, 128, NT, D).transpose(0, 2, 1, 3).reshape(B, N, D)
        )

    ra, rb = unswz(res_a), unswz(res_b)
    attn_a = np.empty((B, N, D), dtype=np.float32)
    attn_b = np.empty((B, N, D), dtype=np.float32)
    for b in range(B):
        attn_a[b, perm[b]] = ra[b].astype(np.float32)
        attn_b[b, perm[b]] = rb[b].astype(np.float32)
    return attn_a, attn_b


def _run(x1, x1_mask, x2, x2_mask, w, bias, **run_kwargs):
    full, MTp, NTp, perm = _prep(x1, x1_mask, x2, x2_mask, w, bias)
    nc = _get_nc(MTp, NTp)
    in_maps = []
    for core in range(NCORES):
        lo, hi = core * BPC, (core + 1) * BPC
        in_maps.append({k: v[lo:hi] for k, v in full.items()})
    res = bass_utils.run_bass_kernel_spmd(
        nc, in_maps, core_ids=list(range(NCORES)), **run_kwargs
    )
    res_a = np.concatenate([res.results[c]["attn_a"] for c in range(NCORES)], axis=0)
    res_b = np.concatenate([res.results[c]["attn_b"] for c in range(NCORES)], axis=0)
    attn_a, attn_b = _unpermute(res_a, res_b, perm)
    return (attn_a, attn_b), res


def kernel(x1, x1_mask, x2, x2_mask, w, bias):
    out, _ = _run(x1, x1_mask, x2, x2_mask, w, bias)
    return out


# revision 13
# speedup vs baseline: 1.1320x; 1.0205x over previous
"""BiAttention Trainium2 kernel (mask-compacted fp16, permuted-n, flat-DMA).

Reference math (per batch; n = m = 1024, d = 512):
    sim[n,m] = (x1*w3) @ x2.T + s1[n] + s2[m] + bias,  s1 = x1@w1, s2 = x2@w2
    row softmax over m with x2-masked columns dropped -> attn_a = P_row @ x2
    col softmax over n with x1-masked rows dropped    -> q2c = P_col.T @ x1
    attn_b = P_row @ q2c

The masks are ~50% dense, so masked positions are compacted away on the host:
  - m is compacted to m' = max unmasked x2 count (128-padded): every
    m-contraction (row softmax, attn_a, attn_b) skips masked columns.
  - n is PERMUTED so unmasked x1 rows come first (outputs are un-permuted on
    the host). The column softmax then only involves the leading n' rows, so
    its numerator EC = ET^T restricted to the first n' columns — obtained
    with cheap PE transposes of ET instead of a second matmul, and the
    q2c contraction shrinks to n' as well.

Kernel formulation (softmax is shift-invariant, so each direction only needs
the logit terms that vary along its own axis):
    ET[m',n] = exp(s3^T[m',n] + lane2[m']),  lane2 = s2 + bias + NEG*pad
        (s1[n] cancels in the row softmax; per-partition ACT exp bias)
    TC[n',m'] = ET^T (PE transpose of the leading n' column tiles)
    rowsum[n] = sum_m' ET[m',n]:
        n < n' : free via DVE accum_out on the TC PSUM->SBUF copies
        n >= n': tiny PE matmul chains against a ones vector
    colsum[m'] = sum_n' TC[n',m']*B[n'],  B = exp(s1)*keep1  (tiny PE chains
        against B; the x1 row mask and col-softmax exp(s1) factor live in B)
    attn_a = (ET.T @ x2') / rowsum
    q2c    = (TC.T @ (B*x1')) / colsum
    attn_b = (ET.T @ q2c) / rowsum
Padded m' columns have ET = 0 (lane2 = NEG), so their (finite, junk) q2c
rows are multiplied by exactly-0 weights in attn_b, matching the reference.
Trailing masked n rows have B = 0, so they drop out of colsum/q2c exactly.

Implementation notes:
  - exp() without max-subtraction: logits are O(+-5) and masking/padding is
    additive -30000 so exp underflows to exactly 0 (exactly representable
    in the fp16 lane2 vector; its ~1e-3 rounding on live logits is far
    inside the error budget).
  - Matmuls run in fp16. PSUM accumulation is fp32.
  - DMA is descriptor-rate limited (~15ns/descriptor, serialized per
    queue), so every transfer is exactly 128 contiguous partition lines:
    inputs are pre-swizzled flat on the host, the small per-row vectors
    (lane2, B) ride as extra columns of the A2 pack, the identity matrix
    rides in the A1 pack, and outputs are staged into one [128, N/128*D]
    tile per output and shipped as two half-tensor DMAs.
  - Outputs fp16 (halves output DMA), upcast + un-permuted on host.
  - Sharding: data-parallel over batch, 2 batches per core, 8 cores.
"""

import sys

import numpy as np

for _p in ("/opt/trn_rl_repo",):
    if _p not in sys.path:
        sys.path.append(_p)

import concourse.bass as bass
import concourse.mybir as mybir
import concourse.tile as tile
from concourse import bass_utils
from concourse.bass import ds, ts
from concourse.tile import ScopedClock

NCORES = 8
B, N, M, D = 16, 1024, 1024, 512
BPC = B // NCORES  # batches per core
NEG = -30000.0  # additive mask: exp(x + NEG) == 0 for |x| < ~100

F32 = mybir.dt.float32
F16 = mybir.dt.float16
MNP = np.float16
ADD = mybir.AluOpType.add

NT = N // 128  # 8 n-tiles (full axis: attn_a/attn_b output rows)
DC = D // 128  # 4 d-chunks
NH = N // 512  # 2 n-halves (PSUM-bank-sized slabs)

# ---------------------------------------------------------------------------
# Workarounds for this walrus build: at most ONE sync wait per instruction.
# ---------------------------------------------------------------------------

_ctr = [0]


def _split_multi_waits(nc):
    """Move extra sync waits onto same-engine InstNoOp carriers inserted
    immediately before the over-subscribed instruction."""
    for f in nc.m.functions:
        for bb in f.blocks:
            insts = bb.instructions
            i = 0
            while i < len(insts):
                inst = insts[i]
                si = getattr(inst, "sync_info", None)
                if si is not None and len(si.on_wait) > 1:
                    waits = list(si.on_wait)
                    carriers = []
                    for w in waits[:-1]:
                        _ctr[0] += 1
                        carriers.append(
                            mybir.InstNoOp(
                                name=f"I-waitsplit-{_ctr[0]}",
                                engine=inst.engine,
                                bass_nofuse=True,
                                sync_info=mybir.SyncInfo(on_wait=[w], on_update=[]),
                            )
                        )
                    inst.sync_info = mybir.SyncInfo(
                        on_wait=[waits[-1]], on_update=list(si.on_update)
                    )
                    insts[i:i] = carriers
                    i += len(carriers)
                i += 1


def _patched_drain_and_barrier(self, tick_clock, wait_clock):
    """TileContext tail drain: carry the global-clock waits on SP nops (the
    Drain opcode can't encode sync waits in this walrus build)."""
    nc = self.nc
    nop_inst = nc.sync.nop(nofuse=True)
    wait_clock.add_sem_waits(nop_inst.ins, ScopedClock({None: tick_clock.global_clock}))
    waits = list(nop_inst.ins.sync_info.on_wait)
    if len(waits) > 1:
        nop_inst.ins.sync_info = mybir.SyncInfo(on_wait=[waits[0]], on_update=[])
        for w in waits[1:]:
            extra = nc.sync.nop(nofuse=True)
            extra.ins.sync_info = mybir.SyncInfo(on_wait=[w], on_update=[])
    nc.sync.drain()
    nc.all_engine_barrier()
    assert self.sems is not None
    popped = nc._tile_sem_poison_stack.pop()
    assert popped is self._sem_poison
    nc.clear_and_free_semaphores(list(self.sems.allocated().values()))


tile.TileContext._drain_and_barrier = _patched_drain_and_barrier

# ---------------------------------------------------------------------------
# Kernel build
# ---------------------------------------------------------------------------

_cache = {}


def _build(MTp, NTp, split_waits=True):
    """MTp/NTp: number of 128-tiles of the compacted m'/permuted n' axes."""
    Mp, Np = MTp * 128, NTp * 128
    # flat pack geometry
    A1W = NH * DC * 512 + 128  # [h0 (2048) | ident (128) | h1 (2048)]
    H0 = 0  # h0 block offset
    IDO = DC * 512  # ident offset (2048)
    H1 = DC * 512 + 128  # h1 block offset (2176)
    A2W = DC * Mp + MTp + NTp  # [x2^T chunks | lane2 | B]
    LNO = DC * Mp  # lane2 offset
    BVO = DC * Mp + MTp  # B offset
    OW = NT * D  # flat output width per partition

    nc = bass.Bass("TRN2", target_bir_lowering=False, debug=False)

    a1h0d = nc.dram_tensor("a1h0", [BPC, 128, H1], F16, kind="ExternalInput").ap()
    a1h1d = nc.dram_tensor("a1h1", [BPC, 128, A1W - H1], F16, kind="ExternalInput").ap()
    a2d = nc.dram_tensor("a2p", [BPC, 128, A2W], F16, kind="ExternalInput").ap()
    x2cd = nc.dram_tensor("x2c", [BPC, 128, MTp * D], F16, kind="ExternalInput").ap()
    x1md = nc.dram_tensor("x1m", [BPC, 128, NTp * D], F16, kind="ExternalInput").ap()
    oad = nc.dram_tensor("attn_a", [BPC, 128, OW], F16, kind="ExternalOutput").ap()
    obd = nc.dram_tensor("attn_b", [BPC, 128, OW], F16, kind="ExternalOutput").ap()

    EXP = mybir.ActivationFunctionType.Exp

    with tile.TileContext(nc) as tc:
        with (
            tc.tile_pool(name="xin", bufs=2) as xin,
            tc.tile_pool(name="amat", bufs=2) as amat,
            tc.tile_pool(name="emat", bufs=2) as emat,
            tc.tile_pool(name="qmat", bufs=2) as qmat,
            tc.tile_pool(name="small", bufs=2) as small,
            tc.tile_pool(name="ostage", bufs=2) as ostage,
            tc.tile_pool(name="mm_ps", bufs=3, space="PSUM") as mm_ps,
            tc.tile_pool(name="acc_ps", bufs=3, space="PSUM") as acc_ps,
            tc.tile_pool(name="sm_ps", bufs=2, space="PSUM") as sm_ps,
        ):
            for b in range(BPC):
                # ---- loads: one contiguous 128-line transfer each ---------
                A1 = amat.tile([128, A1W], F16, tag="A1")  # w3*x1^T | ident
                A2 = amat.tile([128, A2W], F16, tag="A2")  # x2^T | lane2 | B
                nc.sync.dma_start(out=A2[:], in_=a2d[b])
                nc.scalar.dma_start(out=A1[:, 0:H1], in_=a1h0d[b])
                nc.scalar.dma_start(out=A1[:, H1:A1W], in_=a1h1d[b])
                X2C = xin.tile([128, MTp * D], F16, tag="X2C")
                X1M = xin.tile([128, NTp * D], F16, tag="X1M")
                nc.sync.dma_start(out=X2C[:], in_=x2cd[b])
                nc.sync.dma_start(out=X1M[:], in_=x1md[b])
                ident = A1[:, ds(IDO, 128)]

                if b == 0:
                    # after the load triggers: preload the ACT exp table and
                    # warm the PE clock (HAM) with dummy matmuls during the
                    # load wait; without this the first ~3.4us of real
                    # matmuls run at half clock
                    warm = small.tile([128, 2], F32, tag="warm")
                    nc.vector.memset(warm[:], 0.0)
                    nc.scalar.activation(out=warm[:], in_=warm[:], func=EXP)
                    wsb = small.tile([128, 512], F16, tag="wsb")
                    nc.vector.memset(wsb[:], 0.0)
                    wps = mm_ps.tile([128, 512], F32, tag="mm")
                    for _ in range(8):
                        nc.tensor.matmul(
                            wps[:], wsb[:, 0:128], wsb[:], start=True, stop=True
                        )
                    ones = small.tile([128, 1], F16, tag="ones")
                    nc.vector.memset(ones[:], 1.0)
                    epsv = small.tile([128, 1], F32, tag="epsv")
                    nc.vector.memset(epsv[:], 1e-30)

                def a1s(nh, c):
                    return A1[:, ds((H1 if nh else H0) + c * 512, 512)]

                # ---- ET = exp(s3^T + lane2[m'])  [m'-part, n-free] --------
                ET = emat.tile([128, MTp, N], F16, tag="ET")
                for nh in range(NH):
                    for mt in range(MTp):
                        ps = mm_ps.tile([128, 512], F32, tag="mm")
                        for c in range(DC):
                            nc.tensor.matmul(
                                ps[:],
                                A2[:, ds(c * Mp + mt * 128, 128)],
                                a1s(nh, c),
                                start=(c == 0),
                                stop=(c == DC - 1),
                            )
                        nc.scalar.activation(
                            out=ET[:, mt, ds(512 * nh, 512)],
                            in_=ps[:],
                            func=EXP,
                            bias=A2[:, ds(LNO + mt, 1)],
                        )

                # ---- TC = ET^T for the leading n' tiles (PE transpose) ----
                # The DVE copy off PSUM also reduces each row into rtmp:
                # rowsum[n] = sum_m' TC[n, m'] for free (accum_out).
                TC = emat.tile([128, NTp, Mp], F16, tag="TC")
                rtmp = small.tile([128, NT], F32, tag="rtmp")
                RR = small.tile([128, NT], F32, tag="RR")
                for nt in range(NTp):
                    tps = mm_ps.tile([128, Mp], F16, tag="mm")
                    for mt in range(MTp):
                        nc.tensor.transpose(
                            tps[:, ts(mt, 128)],
                            ET[:, mt, ts(nt, 128)],
                            ident,
                        )
                    nc.vector.tensor_scalar(
                        out=TC[:, nt, :],
                        in0=tps[:],
                        scalar1=0.0,
                        scalar2=None,
                        op0=ADD,
                        op1=ADD,
                        accum_out=rtmp[:, nt : nt + 1],
                    )
                # rowsum for the trailing (masked-n) tiles: tiny PE chains
                for nt in range(NTp, NT):
                    rps = sm_ps.tile([128, 1], F32, tag="sm")
                    for mc in range(MTp):
                        nc.tensor.matmul(
                            rps[:],
                            ET[:, mc, ts(nt, 128)],
                            ones[:],
                            start=(mc == 0),
                            stop=(mc == MTp - 1),
                        )
                    nc.vector.tensor_copy(rtmp[:, nt : nt + 1], rps[:])
                nc.vector.reciprocal(RR[:], rtmp[:])

                # ---- attn_a = (ET.T @ x2') / rowsum -----------------------
                SA = ostage.tile([128, OW], F16, tag="sa")
                for nt in range(NT):
                    aps = acc_ps.tile([128, 512], F32, tag="acc")
                    for mc in range(MTp):
                        nc.tensor.matmul(
                            aps[:],
                            ET[:, mc, ts(nt, 128)],
                            X2C[:, ds(mc * D, D)],
                            start=(mc == 0),
                            stop=(mc == MTp - 1),
                        )
                    nc.vector.tensor_scalar_mul(
                        SA[:, ds(nt * D, D)], aps[:], RR[:, nt : nt + 1]
                    )
                    if nt == NT // 2 - 1:
                        nc.sync.dma_start(
                            out=oad[b][:, 0 : OW // 2], in_=SA[:, 0 : OW // 2]
                        )
                nc.sync.dma_start(out=oad[b][:, OW // 2 : OW], in_=SA[:, OW // 2 : OW])

                # ---- colsum[m'] = sum_n' TC*B (tiny PE chains vs B) -------
                # epilogue adds run on ACT: the DVE queue is busy with the
                # attn_a stage scalings and would serialize the chains.
                ctmp = small.tile([128, MTp], F32, tag="ctmp")
                CR = small.tile([128, MTp], F32, tag="CR")
                for mt in range(MTp):
                    cps = sm_ps.tile([128, 1], F32, tag="sm")
                    for nc_ in range(NTp):
                        nc.tensor.matmul(
                            cps[:],
                            TC[:, nc_, ts(mt, 128)],
                            A2[:, ds(BVO + nc_, 1)],
                            start=(nc_ == 0),
                            stop=(nc_ == NTp - 1),
                        )
                    nc.scalar.activation(
                        out=ctmp[:, mt : mt + 1],
                        in_=cps[:],
                        func=mybir.ActivationFunctionType.Identity,
                        bias=epsv[:],
                    )
                nc.vector.reciprocal(CR[:], ctmp[:])

                # ---- q2c = (TC.T @ (B*x1')) / colsum ----------------------
                Q2C = qmat.tile([128, MTp, D], F16, tag="Q2C")
                for mt in range(MTp):
                    qps = acc_ps.tile([128, 512], F32, tag="acc")
                    for nc_ in range(NTp):
                        nc.tensor.matmul(
                            qps[:],
                            TC[:, nc_, ts(mt, 128)],
                            X1M[:, ds(nc_ * D, D)],
                            start=(nc_ == 0),
                            stop=(nc_ == NTp - 1),
                        )
                    nc.scalar.mul(Q2C[:, mt, :], qps[:], CR[:, mt : mt + 1])

                # ---- attn_b = (ET.T @ q2c) / rowsum -----------------------
                SB = ostage.tile([128, OW], F16, tag="sb")
                for nt in range(NT):
                    bps = acc_ps.tile([128, 512], F32, tag="acc")
                    for mc in range(MTp):
                        nc.tensor.matmul(
                            bps[:],
                            ET[:, mc, ts(nt, 128)],
                            Q2C[:, mc, :],
                            start=(mc == 0),
                            stop=(mc == MTp - 1),
                        )
                    nc.vector.tensor_scalar_mul(
                        SB[:, ds(nt * D, D)], bps[:], RR[:, nt : nt + 1]
                    )
                    # ship quarters as they complete, alternating queues, so
                    # the post-last-matmul tail only covers one quarter
                    if nt % 2 == 1 and nt < NT - 1:
                        q = OW // 4
                        k = nt // 2
                        eng = nc.scalar if k % 2 == 0 else nc.sync
                        eng.dma_start(
                            out=obd[b][:, k * q : (k + 1) * q],
                            in_=SB[:, k * q : (k + 1) * q],
                        )
                nc.scalar.dma_start(
                    out=obd[b][:, 3 * (OW // 4) : OW], in_=SB[:, 3 * (OW // 4) : OW]
                )

    if split_waits:
        _split_multi_waits(nc)
    return nc


def _get_nc(MTp, NTp):
    key = (MTp, NTp)
    if key not in _cache:
        _cache[key] = _build(MTp, NTp)
    return _cache[key]


# ---------------------------------------------------------------------------
# Host entry point
# ---------------------------------------------------------------------------


def _prep(x1, x1_mask, x2, x2_mask, w, bias):
    """Host-side marshaling: n-permutation, m-compaction gathers, flat
    SBUF-layout swizzles, fp16 casts, and the fused per-row/col vectors."""
    x1 = np.asarray(x1, dtype=np.float32)
    x2 = np.asarray(x2, dtype=np.float32)
    x1_mask = np.asarray(x1_mask, dtype=bool)
    x2_mask = np.asarray(x2_mask, dtype=bool)
    w = np.asarray(w, dtype=np.float32)
    bias_f = float(np.asarray(bias, dtype=np.float32))

    d = x1.shape[-1]
    w1, w2, w3 = w[:d], w[d : 2 * d], w[2 * d :]
    s1 = np.einsum("bnd,d->bn", x1, w1)
    s2 = np.einsum("bmd,d->bm", x2, w2)
    x1w3 = x1 * w3

    keepm = [np.flatnonzero(~x2_mask[b]) for b in range(B)]
    MTp = max(1, -(-max(len(i) for i in keepm) // 128))
    NTp = max(1, -(-int((~x1_mask).sum(1).max()) // 128))
    Mp, Np = MTp * 128, NTp * 128
    H1 = DC * 512 + 128
    A2W = DC * Mp + MTp + NTp

    perm = np.argsort(x1_mask, axis=1, kind="stable")  # unmasked n first

    a1h0 = np.zeros((B, 128, H1), dtype=MNP)
    a1h1 = np.zeros((B, 128, DC * 512), dtype=MNP)
    a2p = np.zeros((B, 128, A2W), dtype=MNP)
    x2c = np.zeros((B, 128, MTp * D), dtype=MNP)
    x1m = np.zeros((B, 128, NTp * D), dtype=MNP)
    eye = np.eye(128, dtype=MNP)
    for b in range(B):
        im, pi = keepm[b], perm[b]
        a1t = x1w3[b][pi].T.astype(MNP)  # [D, N] permuted n
        # [DC, 128, NH, 512] -> per half [128, DC*512]
        a1q = a1t.reshape(DC, 128, NH, 512)
        a1h0[b, :, : DC * 512] = (
            a1q[:, :, 0].transpose(1, 0, 2).reshape(128, DC * 512)
        )
        a1h0[b, :, DC * 512 :] = eye
        a1h1[b] = a1q[:, :, 1].transpose(1, 0, 2).reshape(128, DC * 512)
        a2t = np.zeros((d, Mp), dtype=MNP)
        a2t[:, : len(im)] = x2[b, im].T
        a2p[b, :, : DC * Mp] = a2t.reshape(DC, 128, Mp).transpose(1, 0, 2).reshape(
            128, DC * Mp
        )
        lane2 = np.full(Mp, NEG, dtype=np.float32)
        lane2[: len(im)] = s2[b, im] + bias_f
        a2p[b, :, DC * Mp : DC * Mp + MTp] = lane2.reshape(MTp, 128).T.astype(MNP)
        bvp = np.exp(s1[b][pi[:Np]]) * (~x1_mask[b][pi[:Np]])
        a2p[b, :, DC * Mp + MTp :] = bvp.reshape(NTp, 128).T.astype(MNP)
        xc = np.zeros((Mp, d), dtype=MNP)
        xc[: len(im)] = x2[b, im]
        x2c[b] = xc.reshape(MTp, 128, d).transpose(1, 0, 2).reshape(128, MTp * d)
        xm = (x1[b][pi[:Np]] * bvp[:, None]).astype(MNP)
        x1m[b] = xm.reshape(NTp, 128, d).transpose(1, 0, 2).reshape(128, NTp * d)

    tensors = {
        "a1h0": a1h0,
        "a1h1": a1h1,
        "a2p": a2p,
        "x2c": x2c,
        "x1m": x1m,
    }
    return tensors, MTp, NTp, perm


def _unpermute(res_a, res_b, perm):
    # res [B, 128, NT*D] -> [B, N, D] (row n' = nt*128+p), then un-permute n
    def unswz(r):
        return (
            r.reshape(B, 128, NT, D).transpose(0, 2, 1, 3).reshape(B, N, D)
        )

    ra, rb = unswz(res_a), unswz(res_b)
    attn_a = np.empty((B, N, D), dtype=np.float32)
    attn_b = np.empty((B, N, D), dtype=np.float32)
    for b in range(B):
        attn_a[b, perm[b]] = ra[b].astype(np.float32)
        attn_b[b, perm[b]] = rb[b].astype(np.float32)
    return attn_a, attn_b


def _run(x1, x1_mask, x2, x2_mask, w, bias, **run_kwargs):
    full, MTp, NTp, perm = _prep(x1, x1_mask, x2, x2_mask, w, bias)
    nc = _get_nc(MTp, NTp)
    in_maps = []
    for core in range(NCORES):
        lo, hi = core * BPC, (core + 1) * BPC
        in_maps.append({k: v[lo:hi] for k, v in full.items()})
    res = bass_utils.run_bass_kernel_spmd(
        nc, in_maps, core_ids=list(range(NCORES)), **run_kwargs
    )
    res_a = np.concatenate([res.results[c]["attn_a"] for c in range(NCORES)], axis=0)
    res_b = np.concatenate([res.results[c]["attn_b"] for c in range(NCORES)], axis=0)
    attn_a, attn_b = _unpermute(res_a, res_b, perm)
    return (attn_a, attn_b), res


def kernel(x1, x1_mask, x2, x2_mask, w, bias):
    out, _ = _run(x1, x1_mask, x2, x2_mask, w, bias)
    return out
